# revision 13
# baseline (speedup 1.0000x reference)
"""OHEM loss Trainium2 kernel.

Problem: B=32 images, N=131072 priors, L=10 gt boxes per image, f32.
Sharding: pure data parallel, 4 images per NeuronCore across 8 cores.

Device algorithm per image (all N-sized work on device):
  1. Stream the 5 prior fields (cx, cy, w, h, logit) in a partition-wrapped
     layout: prior j lives at SBUF (partition j%128, free j//128).
  2. Cheap conservative candidate filter in fp16: a prior can have iou>0
     with label l only if  max(|du|,|dv|) <= (w+h)/2 + (lw+lh)/2  where
     u=cx+cy, v=cx-cy (the |dx|+|dy| diamond window).  acc = min over
     labels of (max(|du|,|dv|) - (lw+lh)/2); candidate iff acc-16 <= (w+h)/2
     (16 = fp16 rounding slack; filter is conservative, never drops a true
     overlap pair).  ~3-4.5K candidates per image out of 131072.
  3. Compact candidates per partition: cumsum of mask -> local_scatter of
     the free-index -> global row offsets -> indirect DMA gather of 32-byte
     per-prior records (cx,cy,w,h,logit,j,0,0) from a DRAM table.
  4. Exact phase (f32) on the gathered [128, 64] candidates x 10 labels:
     inter = relu(min(x2,lx2)-max(x1,lx1)) * relu(...y...)
     iou >= 0.5  <=>  3*inter - (area+la) >= 0      (no division)
     iou >  0.35 <=>  (27/7)*inter - (area+la) > 0
     Reduce per (partition, label): pos-passer count, sum of (j+1) over
     pos passers, neg-passer count.
  5. Also: global min/max logit partials and the raw logits of priors
     0..1023 (for the first-10-low-conf-prior selection).
Host tail (O(L) per image): assemble passers, exact-f32 iou argmax,
OHEM negative selection, smooth-L1 regression, final scalars.
"""
import numpy as np

B, N, L = 32, 131072, 10
NCORES = 8
IPC = B // NCORES          # images per core
F = N // 128               # free dim of field tiles
CAP = 48                   # candidate slots per partition (f32 filter max seen: 41)
NUM_NEG_RATIO = 3
UP_CONF, LO_CONF = 0.8, 0.3
UP_IOU, LO_IOU = 0.5, 0.35
REG_COEFF = 1.0
MAX_OBJS = 10
TU = float(np.log(np.float32(0.8) / np.float32(0.2)))   # logit threshold conf>0.8
TL = float(np.log(np.float32(0.3) / np.float32(0.7)))   # logit threshold conf<0.3
SLACK = 1.0                # rounding slack for the f32 diamond filter
RES_W = 40                 # output row: 10 cntU | 10 jsum | 10 cntL | 8 logit pfx | min | max

_prog_cache = {}


def build_program():
    """Build the per-core Bass program (4 images). Returns the compiled Bacc."""
    if "nc" in _prog_cache:
        return _prog_cache["nc"]
    import concourse.bass as bass
    import concourse.mybir as mybir
    import concourse.tile as tile
    from concourse import library_config
    import concourse.bacc as bacc

    ALU = mybir.AluOpType
    AF = mybir.ActivationFunctionType
    f32 = mybir.dt.float32
    i16 = mybir.dt.int16

    nc = bacc.Bacc("TRN2", target_bir_lowering=False, debug=False, num_devices=NCORES)
    fields_d = nc.dram_tensor("fields", [IPC, 5, 128, F], f32, kind="ExternalInput")
    labc_d = nc.dram_tensor("labc", [IPC, 128, 8 * L], f32, kind="ExternalInput")
    iof1_d = nc.dram_tensor("iof1", [128, F], i16, kind="ExternalInput")
    pmf_d = nc.dram_tensor("pmf", [128, 1], f32, kind="ExternalInput")
    out_d = nc.dram_tensor("res", [IPC, 128, RES_W], f32, kind="ExternalOutput")

    with tile.TileContext(nc) as tc:
        with tc.tile_pool(name="const", bufs=1) as cpool:
            nc.gpsimd.load_library(library_config.local_scatter)
            ioF1 = cpool.tile([128, F], i16, name="ioF1")
            nc.sync.dma_start(ioF1[:], iof1_d.ap())
            pmF = cpool.tile([128, 1], f32, name="pmF")
            nc.sync.dma_start(pmF[:], pmf_d.ap())

            with tc.tile_pool(name="work", bufs=2) as pool, \
                 tc.tile_pool(name="pair", bufs=1) as ppool:
                for img in range(IPC):
                    # ---- stream in fields + label constants ----
                    cxT = pool.tile([128, F], f32, name="cxT", tag="cxT")
                    cyT = pool.tile([128, F], f32, name="cyT", tag="cyT")
                    wT = pool.tile([128, F], f32, name="wT", tag="wT")
                    hT = pool.tile([128, F], f32, name="hT", tag="hT")
                    lgT = pool.tile([128, F], f32, name="lgT", tag="lgT")
                    for t, ch in ((cxT, 0), (cyT, 1), (wT, 2), (hT, 3), (lgT, 4)):
                        nc.sync.dma_start(t[:], fields_d.ap()[img, ch])
                    labT = pool.tile([128, 8 * L], f32, name="labT", tag="labT")
                    nc.sync.dma_start(labT[:], labc_d.ap()[img])
                    # label cols: 0:-lu 1:-lv 2:-R 3:lx1 4:lx2 5:ly1 6:ly2 7:la
                    lcol = lambda c, l: labT[:, c * L + l:c * L + l + 1]
                    lrow = lambda c: labT[:, c * L:(c + 1) * L]

                    # ---- per-prior precompute (f32) ----
                    uT = pool.tile([128, F], f32, name="uT", tag="uT")
                    vT = pool.tile([128, F], f32, name="vT", tag="vT")
                    nc.vector.tensor_tensor(uT[:], cxT[:], cyT[:], ALU.add)
                    nc.vector.tensor_tensor(vT[:], cxT[:], cyT[:], ALU.subtract)
                    gh2T = pool.tile([128, F], f32, name="gh2T", tag="gh2T")
                    nc.vector.tensor_tensor(gh2T[:], wT[:], hT[:], ALU.add)
                    lgmin = pool.tile([128, 1], f32, name="lgmin", tag="lgmin")
                    lgmax = pool.tile([128, 1], f32, name="lgmax", tag="lgmax")
                    nc.vector.tensor_reduce(lgmin[:], lgT[:], mybir.AxisListType.X, ALU.min)
                    nc.vector.tensor_reduce(lgmax[:], lgT[:], mybir.AxisListType.X, ALU.max)

                    # ---- hot loop: acc = min_l (max(|u-lu|,|v-lv|) - R_l) ----
                    accA = pool.tile([128, F], f32, name="accA", tag="accA")
                    accB = pool.tile([128, F], f32, name="accB", tag="accB")
                    accs = [accA, accB]
                    for l in range(L):
                        aT = pool.tile([128, F], f32, name="aT", tag="aT", bufs=3)
                        bT = pool.tile([128, F], f32, name="bT", tag="bT", bufs=3)
                        cT = pool.tile([128, F], f32, name="cT", tag="cT", bufs=2)
                        nc.scalar.activation(aT[:], uT[:], AF.Abs,
                                             bias=lcol(0, l), scale=1.0)
                        nc.scalar.activation(bT[:], vT[:], AF.Abs,
                                             bias=lcol(1, l), scale=1.0)
                        nc.vector.tensor_tensor(cT[:], aT[:], bT[:], ALU.max)
                        if l == 0:
                            nc.vector.tensor_scalar(accs[0][:], cT[:], lcol(2, l),
                                                    None, ALU.add)
                        else:
                            nc.vector.scalar_tensor_tensor(
                                accs[l % 2][:], cT[:], lcol(2, l),
                                accs[(l + 1) % 2][:], ALU.add, ALU.min)
                    accT = accs[(L - 1) % 2]

                    # ---- candidate mask -> positions ----
                    m01 = pool.tile([128, F], f32, name="m01", tag="m01")
                    nc.vector.scalar_tensor_tensor(m01[:], accT[:], 2.0, gh2T[:],
                                                   ALU.mult, ALU.is_le)
                    incT = pool.tile([128, F], f32, name="incT", tag="incT")
                    nc.vector.tensor_tensor_scan(incT[:], m01[:], m01[:], 0.0,
                                                 ALU.add, ALU.bypass)
                    posF = pool.tile([128, F], f32, name="posF", tag="posF")
                    nc.vector.tensor_tensor(posF[:], incT[:], m01[:], ALU.mult)
                    posI = pool.tile([128, F], i16, name="posI", tag="posI")
                    nc.vector.tensor_scalar_add(posI[:], posF[:], -1.0)
                    # doubled positions for the f32(=2x int16) field scatters
                    p2 = pool.tile([128, F, 2], i16, name="p2", tag="p2")
                    nc.vector.tensor_scalar(p2[:, :, 0], posF[:], 2.0, -2.0,
                                            ALU.mult, ALU.add)
                    nc.scalar.activation(p2[:, :, 1], posF[:], AF.Copy,
                                         bias=-1.0, scale=2.0)
                    p2f = p2[:].rearrange("p a b -> p (a b)")
                    # upper/lower class field: +1 upper, -1 lower, 0 neither
                    lo01 = pool.tile([128, F], f32, name="lo01", tag="lo01")
                    nc.vector.tensor_scalar(lo01[:], lgT[:], TL, None, ALU.is_lt)
                    ulm = pool.tile([128, F], i16, name="ulm", tag="ulm")
                    nc.vector.scalar_tensor_tensor(ulm[:], lgT[:], TU, lo01[:],
                                                   ALU.is_gt, ALU.subtract)

                    # ---- per-partition compaction via local_scatter ----
                    fv1 = pool.tile([128, CAP], i16, name="fv1", tag="fv1")
                    nc.gpsimd.local_scatter(fv1[:], ioF1[:], posI[:], channels=128,
                                            num_elems=CAP, num_idxs=F)
                    gulm = pool.tile([128, CAP], i16, name="gulm", tag="gulm")
                    nc.gpsimd.local_scatter(gulm[:], ulm[:], posI[:], channels=128,
                                            num_elems=CAP, num_idxs=F)
                    g = {}
                    for nm, src in (("gcx", cxT), ("gcy", cyT), ("gw", wT), ("gh", hT)):
                        d16 = pool.tile([128, 2 * CAP], i16,
                                        name=f"d16{nm}", tag=f"d16{nm}")
                        nc.gpsimd.local_scatter(d16[:], src[:].bitcast(i16),
                                                p2f, channels=128,
                                                num_elems=2 * CAP, num_idxs=2 * F)
                        g[nm] = d16[:].bitcast(f32)

                    # ---- small candidate-derived tiles (f32, [128, CAP]) ----
                    fvF = pool.tile([128, CAP], f32, name="fvF", tag="fvF")
                    nc.vector.tensor_copy(fvF[:], fv1[:])
                    gj = pool.tile([128, CAP], f32, name="gj", tag="gj")
                    nc.vector.scalar_tensor_tensor(
                        gj[:], fvF[:], 128.0, pmF[:].broadcast_to([128, CAP]),
                        ALU.mult, ALU.add)
                    gj1 = pool.tile([128, CAP], f32, name="gj1", tag="gj1")
                    nc.vector.tensor_scalar_add(gj1[:], gj[:], 1.0)
                    gulmF = pool.tile([128, CAP], f32, name="gulmF", tag="gulmF")
                    nc.vector.tensor_copy(gulmF[:], gulm[:])
                    gup = pool.tile([128, CAP], f32, name="gup", tag="gup")
                    glo = pool.tile([128, CAP], f32, name="glo", tag="glo")
                    nc.vector.tensor_scalar(gup[:], gulmF[:], 0.5, None, ALU.is_ge)
                    nc.vector.tensor_scalar(glo[:], gulmF[:], -0.5, None, ALU.is_le)
                    gx1 = pool.tile([128, CAP], f32, name="gx1", tag="gx1")
                    gx2 = pool.tile([128, CAP], f32, name="gx2", tag="gx2")
                    gy1 = pool.tile([128, CAP], f32, name="gy1", tag="gy1")
                    gy2 = pool.tile([128, CAP], f32, name="gy2", tag="gy2")
                    nc.vector.scalar_tensor_tensor(gx1[:], g["gw"], -0.5, g["gcx"], ALU.mult, ALU.add)
                    nc.vector.scalar_tensor_tensor(gx2[:], g["gw"], 0.5, g["gcx"], ALU.mult, ALU.add)
                    nc.vector.scalar_tensor_tensor(gy1[:], g["gh"], -0.5, g["gcy"], ALU.mult, ALU.add)
                    nc.vector.scalar_tensor_tensor(gy2[:], g["gh"], 0.5, g["gcy"], ALU.mult, ALU.add)
                    gar = pool.tile([128, CAP], f32, name="gar", tag="gar")
                    nc.vector.tensor_tensor(gar[:], g["gw"], g["gh"], ALU.mult)

                    # ---- exact phase, label-batched [128, L, CAP] (f32) ----
                    bc = lambda t: t[:].unsqueeze(1).broadcast_to([128, L, CAP])
                    lb = lambda c: lrow(c).unsqueeze(2).broadcast_to([128, L, CAP])
                    def PT(nm):
                        return ppool.tile([128, L, CAP], f32, name=nm, tag=nm)
                    tlx = PT("tlx"); brx = PT("brx"); wx = PT("wx"); wy = PT("wy")
                    q = PT("q"); S = PT("S"); tv = PT("tv"); p01 = PT("p01"); jp = PT("jp")
                    nc.vector.tensor_tensor(tlx[:], bc(gx1), lb(3), ALU.max)
                    nc.vector.tensor_tensor(brx[:], bc(gx2), lb(4), ALU.min)
                    nc.vector.tensor_tensor(wx[:], brx[:], tlx[:], ALU.subtract)
                    nc.scalar.activation(wx[:], wx[:], AF.Relu)
                    nc.vector.tensor_tensor(tlx[:], bc(gy1), lb(5), ALU.max)
                    nc.vector.tensor_tensor(brx[:], bc(gy2), lb(6), ALU.min)
                    nc.vector.tensor_tensor(wy[:], brx[:], tlx[:], ALU.subtract)
                    nc.scalar.activation(wy[:], wy[:], AF.Relu)
                    nc.vector.tensor_tensor(q[:], wx[:], wy[:], ALU.mult)
                    nc.vector.tensor_tensor(S[:], bc(gar), lb(7), ALU.add)
                    cntU = pool.tile([128, L], f32, name="cntU", tag="cntU")
                    jsum = pool.tile([128, L], f32, name="jsum", tag="jsum")
                    cntL = pool.tile([128, L], f32, name="cntL", tag="cntL")
                    nc.vector.scalar_tensor_tensor(tv[:], q[:], 3.0, S[:], ALU.mult, ALU.subtract)
                    nc.vector.scalar_tensor_tensor(p01[:], tv[:], 0.0, bc(gup),
                                                   ALU.is_ge, ALU.mult)
                    nc.vector.tensor_reduce(cntU[:], p01[:], mybir.AxisListType.X, ALU.add)
                    nc.vector.tensor_tensor(jp[:], p01[:], bc(gj1), ALU.mult)
                    nc.vector.tensor_reduce(jsum[:], jp[:], mybir.AxisListType.X, ALU.add)
                    nc.vector.scalar_tensor_tensor(tv[:], q[:], 27.0 / 7.0, S[:], ALU.mult, ALU.subtract)
                    nc.vector.scalar_tensor_tensor(p01[:], tv[:], 0.0, bc(glo),
                                                   ALU.is_gt, ALU.mult)
                    nc.vector.tensor_reduce(cntL[:], p01[:], mybir.AxisListType.X, ALU.add)

                    # ---- results out ----
                    nc.sync.dma_start(out_d.ap()[img, :, 0:L], cntU[:])
                    nc.sync.dma_start(out_d.ap()[img, :, L:2 * L], jsum[:])
                    nc.sync.dma_start(out_d.ap()[img, :, 2 * L:3 * L], cntL[:])
                    nc.sync.dma_start(out_d.ap()[img, :, 30:38], lgT[:, 0:8])
                    nc.sync.dma_start(out_d.ap()[img, :, 38:39], lgmin[:])
                    nc.sync.dma_start(out_d.ap()[img, :, 39:40], lgmax[:])

    nc.compile()
    _prog_cache["nc"] = nc
    return nc


def prep_core_inputs(labels, outputs, core):
    """Host-side layout prep for one core's 4 images (layout only + O(L) label math)."""
    ins = {}
    imgs = range(core * IPC, (core + 1) * IPC)
    fields = np.empty((IPC, 5, 128, F), np.float32)
    labc = np.empty((IPC, 128, 8 * L), np.float32)
    for i, b in enumerate(imgs):
        ob = np.ascontiguousarray(outputs[b])            # [N, 5] f32
        for ch in range(5):
            fields[i, ch] = ob[:, ch].reshape(F, 128).T  # prior j -> (j%128, j//128)
        lab = labels[b].astype(np.float32)               # [10, 4]
        lcx, lcy, lw, lh = lab[:, 0], lab[:, 1], lab[:, 2], lab[:, 3]
        zero = lh == 0.0
        last_idx = int(np.argmax(zero)) if zero.any() else L
        gt_valid = np.arange(L) < last_idx
        row = np.empty(8 * L, np.float32)
        row[0:10] = -(lcx + lcy)                         # -lu (ACT Abs bias)
        row[10:20] = -(lcx - lcy)                        # -lv
        # slack folded into R; invalid gts windowed out of the filter
        row[20:30] = np.where(gt_valid, -((lw + lh) * 0.5 + SLACK),
                              np.float32(30000.0))
        row[30:40] = lcx - lw * 0.5                      # lx1
        row[40:50] = lcx + lw * 0.5                      # lx2
        row[50:60] = lcy - lh * 0.5                      # ly1
        row[60:70] = lcy + lh * 0.5                      # ly2
        row[70:80] = lw * lh                             # la
        labc[i] = row[None, :]
    ins["fields"] = fields
    ins["labc"] = labc
    ins["iof1"] = np.broadcast_to(np.arange(1, F + 1, dtype=np.int16)[None, :],
                                  (128, F)).copy()
    ins["pmf"] = (np.arange(128, dtype=np.float32) - 128.0).reshape(128, 1)
    return ins


def _softplus(x):
    return np.logaddexp(np.float32(0.0), x.astype(np.float32)).astype(np.float32)


def _iou_f32(box, lab1):
    """Reference-style f32 iou of prior boxes [K,4] vs one label [4]."""
    box = box.astype(np.float32); lab1 = lab1.astype(np.float32)
    tlx = np.maximum(box[:, 0] - box[:, 2] * 0.5, lab1[0] - lab1[2] * 0.5)
    brx = np.minimum(box[:, 0] + box[:, 2] * 0.5, lab1[0] + lab1[2] * 0.5)
    tly = np.maximum(box[:, 1] - box[:, 3] * 0.5, lab1[1] - lab1[3] * 0.5)
    bry = np.minimum(box[:, 1] + box[:, 3] * 0.5, lab1[1] + lab1[3] * 0.5)
    en = ((tlx < brx) & (tly < bry)).astype(np.float32)
    ai = ((brx - tlx) * (bry - tly) * en).astype(np.float32)
    aa = box[:, 2] * box[:, 3]
    ab = lab1[2] * lab1[3]
    return (ai / (aa + ab - ai)).astype(np.float32)


def host_tail(labels, outputs, core_results):
    """Combine per-image device outputs into the final 4-vector (f32)."""
    conf_losses = np.zeros(B, np.float32)
    reg_losses = np.zeros(B, np.float32)
    nposs = np.zeros(B, np.int64)
    for core in range(NCORES):
        res = core_results[core]                      # [IPC, 128, RES_W]
        for i in range(IPC):
            b = core * IPC + i
            r = res[i]
            lab = labels[b].astype(np.float32)
            ob = outputs[b]
            logit = ob[:, 4].astype(np.float32)
            cntU, jsum, cntL = r[:, 0:L], r[:, L:2 * L], r[:, 2 * L:3 * L]
            lgpfx = r[:, 30:38]
            any_up = r[:, 39].max() > np.float32(TU)
            any_lo = r[:, 38].min() < np.float32(TL)
            # gt validity
            zero = lab[:, 3] == 0.0
            last_idx = int(np.argmax(zero)) if zero.any() else L
            gt_valid = np.arange(L) < last_idx
            # positive side: passers per label
            pos_sel = np.zeros(L, bool)
            pos_prior = np.zeros(L, np.int64)
            for l in range(L):
                tot = cntU[:, l].sum()
                if tot < 0.5:
                    continue
                pas = []
                for p in np.nonzero(cntU[:, l] > 0.5)[0]:
                    c = cntU[p, l]
                    if c < 1.5:
                        pas.append(int(round(float(jsum[p, l]))) - 1)
                    else:
                        # >=2 passers collided in partition p: recompute locally
                        sub = ob[p::128]
                        iou = _iou_f32(sub[:, :4], lab[l])
                        m = (sub[:, 4] > np.float32(TU)) & (iou >= np.float32(UP_IOU))
                        pas.extend((np.nonzero(m)[0] * 128 + p).tolist())
                pas = sorted(set(pas))
                iou = _iou_f32(ob[pas, :4], lab[l])
                mx = iou.max()
                pos_sel[l] = True          # any passer => max iou >= 0.5
                pos_prior[l] = pas[int(np.argmax(iou == mx))]
            gt_sel = gt_valid & pos_sel
            num_pos = int(gt_sel.sum())
            valid = (last_idx <= MAX_OBJS) and any_up and any_lo and (num_pos > 0)
            # first 10 low-conf priors (device supplies logits of priors 0..1023)
            pfx = lgpfx.T.reshape(-1)                 # j order 0..1023
            low_idx = np.nonzero(pfx < np.float32(TL))[0]
            if len(low_idx) < L:
                # fallback: chunked scan of the full logit array (never hit in practice)
                low_idx = []
                start = 0
                while len(low_idx) < L and start < N:
                    chunk = logit[start:start + 4096]
                    low_idx.extend((np.nonzero(chunk < np.float32(TL))[0] + start).tolist())
                    start += 4096
                low_idx = np.array(low_idx[:L], np.int64)
                first_lower = np.zeros(L, np.int64)
                first_lower[:len(low_idx)] = low_idx
            else:
                first_lower = low_idx[:L].astype(np.int64)
            # negative side
            neg_cand = gt_valid & (cntL.sum(axis=0) < 0.5)
            fl_logit = logit[first_lower]
            conf_fl = (np.float32(1.0) / (np.float32(1.0) + np.exp(-fl_logit))).astype(np.float32)
            cand_conf = np.where(neg_cand, conf_fl, -np.inf).astype(np.float32)
            order = np.argsort(-cand_conf, kind="stable")
            rank = np.argsort(order, kind="stable")
            neg_sel = neg_cand & (rank < NUM_NEG_RATIO * num_pos)
            # losses (f32, same formulas as the reference)
            conf_pos = np.where(gt_sel, _softplus(-logit[pos_prior]), np.float32(0)).sum(dtype=np.float32)
            conf_neg = np.where(neg_sel, _softplus(fl_logit), np.float32(0)).sum(dtype=np.float32)
            conf_loss = np.float32(conf_pos + conf_neg)
            pred = ob[pos_prior, :4].astype(np.float32)
            d = pred - lab[:, :4]
            ad = np.abs(d)
            sl1 = np.where(ad < 1.0, np.float32(0.5) * d * d, ad - np.float32(0.5)).astype(np.float32)
            ml = np.max(lab[:, 2:4], axis=1, keepdims=True)
            ml = np.where(ml == 0.0, np.float32(1.0), ml)
            reg_loss = np.where(gt_sel[:, None], sl1 / ml, np.float32(0)).sum(dtype=np.float32)
            vf = np.float32(1.0 if valid else 0.0)
            conf_losses[b] = vf * conf_loss
            reg_losses[b] = vf * reg_loss
            nposs[b] = num_pos if valid else 0
    pos_selected = int(nposs.sum())
    conf_loss = conf_losses.sum(dtype=np.float32)
    reg_loss = reg_losses.sum(dtype=np.float32)
    pos_f = np.float32(max(pos_selected, 1))
    ok = pos_selected >= 1
    total = np.float32((conf_loss + np.float32(REG_COEFF) * reg_loss) / pos_f) if ok else np.float32(0)
    c = np.float32(conf_loss / pos_f) if ok else np.float32(0)
    rr = np.float32(reg_loss / pos_f) if ok else np.float32(0)
    p = np.float32(pos_selected) if ok else np.float32(0)
    return np.array([total, c, rr, p], np.float32)


def kernel(labels, outputs, _trace=False):
    labels = np.asarray(labels)
    outputs = np.asarray(outputs)
    nc = build_program()
    from concourse import bass_utils
    in_maps = [prep_core_inputs(labels, outputs, core) for core in range(NCORES)]
    kw = dict(trace=True) if _trace else {}
    res = bass_utils.run_bass_kernel_spmd(nc, in_maps, core_ids=list(range(NCORES)), **kw)
    core_results = [res.results[core]["res"] for core in range(NCORES)]
    out = host_tail(labels, outputs, core_results)
    if _trace:
        return out, res
    return out
